# revision 3
# baseline (speedup 1.0000x reference)
"""Sharded multi-head attention for TRN2 (8 NeuronCores).

Problem: B=4, H=16, S=2048, DK=64 attention with boolean mask [B,1,S,S]
(True entries masked out).  Core c handles batch c//2, heads
(c%2)*8 .. (c%2)*8+8, processed in PAIRS sharing the PE array.

The kernel is organized around keeping the Scalar (ACT) engine — the
bottleneck (exp runs at 1 elem/cycle/lane @ 1.2 GHz, ~262K cycles/core
of pure element work) — saturated with large ACTIVATEs:

  - PSUM layout: score "slots" are per-(iteration, head) [128, 512] f32
    (one bank).  A "batch" super-tile = [128, 3, 512] f32 (3 banks).
    Two batch bufs (6 banks) + single-buffered acc pair (2 banks) = 8.
  - exp runs once per 3-slot batch (FD=1536), halving per-instruction
    overhead vs per-iteration exps.  The 6-bank ring gives each batch a
    full opposite-exp shadow to refill its QK scores: near-zero ACT idle
    in steady state.
  - macro pipeline (macro = 3 iterations = 2 exp batches): per macro the
    PE issues [3 QK slot-matmuls][3 more][6 PV matmuls] so the QK<->PV
    array transitions (full-row LDWEIGHTS can't overlap cross-row-group
    matmuls) amortize, and both QK refills are issued BEFORE the PV
    block — the in-order PE queue must never park a refill behind PVs.
  - QK pairs run concurrently via row tile_position (0,0)/(64,0), each
    head's d=64 contraction in its own row group.
  - mask multiply on DVE (bf16 2x): one TT for the batch-aligned
    iteration right after the first exp, one 2-iteration TT after the
    second (keep slice broadcast over heads via a stride-0 AP dim, and
    over consecutive kt via a 4D AP).
  - PV: acc[v,q] += V'[k,v]^T w per head, V' = [V | ones] so PSUM row 64
    accumulates the softmax denominators.  acc is single-buffered; a
    fast DVE drain (PSUM->SBUF f32), issued inline right after the
    kt=15 PVs, frees the banks before the next block's kt=0 PV needs
    them, keeping the PE-queue bubble at block boundaries small.
  - epilogue off the critical path: the sums row moves to partition 0
    with a tiny SBUF->SBUF DMA (engines can't shift partitions, DMA
    can; the last two blocks use the by-then-idle ACT engine instead),
    reciprocal_approx_fast, gpsimd partition_broadcast, normalize
    multiply, f32 DMA out in [d, q] layout (host un-transposes).
  - a dummy 8-element exp is issued first so the ~2.7us ACT table load
    overlaps the input DMAs; K arrives before Q's later chunks; keep
    mask slices stream in one kt-triple per macro on the gpsimd queue.
"""

import numpy as np
import ml_dtypes
from contextlib import ExitStack

import concourse.bass as bass
import concourse.tile as tile
from concourse import bacc, mybir
from concourse.bass_utils import run_bass_kernel_spmd

B, H, S, DK = 4, 16, 2048, 64
N_CORES = 8
HPC = (B * H) // N_CORES  # heads per core = 8
NPAIR = HPC // 2

P = 128            # k-tile size / partition count
NKT = S // P       # 16 k tiles
QCH = 512          # q chunk per head (slot = [128, 512] f32 = 1 PSUM bank)
NQ = S // QCH      # 4 q chunks

NITER = NPAIR * NQ * NKT          # 256 iterations, flat (pr, qc, kt)
NSLOT = 2 * NITER                 # 512 head-slots
NMACRO = (NITER + 2) // 3         # 86 macros of 3 iterations
SPB = 3                           # slots per exp batch (3 PSUM banks)
NBATCH = 2 * NMACRO


def batch_of_slot(s):
    return divmod(s, SPB)


def batch_slots(bb):
    s0 = SPB * bb
    ns = min(SPB, NSLOT - s0)
    return s0, ns

BF16 = mybir.dt.bfloat16
F32 = mybir.dt.float32
BF = ml_dtypes.bfloat16


def it_of(i):
    pr, r = divmod(i, NQ * NKT)
    qc, kt = divmod(r, NKT)
    return pr, qc, kt


def build_nc():
    nc = bacc.Bacc(None, target_bir_lowering=False)
    # qkt[pair, 0] = [Q_A^T ; Q_B^T] stacked on partitions, [pair, 1] = K
    qkt_ext = nc.declare_dram_parameter("qkt", [NPAIR, 2, P, S], BF16, isOutput=False)
    # vp[h, p, t, :] = [V[h, t*128+p, :], 1.0]
    vp_ext = nc.declare_dram_parameter("vp", [HPC, P, NKT, DK + 1], BF16, isOutput=False)
    # keep[p, t, q] = not mask[q, t*128+p]
    keep_ext = nc.declare_dram_parameter("keep", [P, NKT, S], BF16, isOutput=False)
    # out_T[h, d, q] f32 (host un-transposes)
    out_ext = nc.declare_dram_parameter("outT", [HPC, DK, S], F32, isOutput=True)

    with tile.TileContext(nc) as tc, ExitStack() as ctx:
        singles = ctx.enter_context(tc.tile_pool(name="singles", bufs=1))
        qk_pool = ctx.enter_context(tc.tile_pool(name="qk", bufs=2))
        v_pool = ctx.enter_context(tc.tile_pool(name="vpool", bufs=2))
        w_pool = ctx.enter_context(tc.tile_pool(name="wp", bufs=9))
        ep_pool = ctx.enter_context(tc.tile_pool(name="ep", bufs=3))
        sc_ps = ctx.enter_context(tc.tile_pool(name="scps", bufs=2, space="PSUM"))
        acc_ps = ctx.enter_context(tc.tile_pool(name="accps", bufs=1, space="PSUM"))

        # dummy exp first: forces the ACT table load to overlap input DMAs
        dummy = singles.tile([1, 8], F32)
        nc.vector.memset(dummy, 0.0)
        nc.scalar.activation(dummy, dummy, mybir.ActivationFunctionType.Exp)

        keep_sb = singles.tile([P, NKT, S], BF16)

        pair_tiles = {}

        def get_pair(pair):
            if pair not in pair_tiles:
                hA, hB = 2 * pair, 2 * pair + 1
                qT2 = qk_pool.tile([P, S], BF16, tag="qT2", name=f"qT2_{pair}")
                kT2 = qk_pool.tile([P, S], BF16, tag="kT2", name=f"kT2_{pair}")
                # startup-critical order: K chunk 0 + Q chunk 0 (prologue
                # QKs), V (PVs from macro 1), remaining K (kt>=4), then the
                # remaining Q chunks (qc>=1)
                C4 = S // 4
                nc.sync.dma_start(out=kT2[:, 0:C4], in_=qkt_ext[pair, 1, :, 0:C4])
                nc.sync.dma_start(out=qT2[:, 0:C4], in_=qkt_ext[pair, 0, :, 0:C4])
                vpA = v_pool.tile([P, NKT, DK + 1], BF16, tag="vpA", name=f"vpA_{pair}")
                vpB = v_pool.tile([P, NKT, DK + 1], BF16, tag="vpB", name=f"vpB_{pair}")
                nc.sync.dma_start(out=vpA, in_=vp_ext[hA])
                nc.sync.dma_start(out=vpB, in_=vp_ext[hB])
                for c4 in range(1, 4):
                    cs = slice(c4 * C4, (c4 + 1) * C4)
                    nc.sync.dma_start(out=kT2[:, cs], in_=qkt_ext[pair, 1, :, cs])
                for c4 in range(1, 4):
                    cs = slice(c4 * C4, (c4 + 1) * C4)
                    nc.sync.dma_start(out=qT2[:, cs], in_=qkt_ext[pair, 0, :, cs])
                pair_tiles[pair] = (qT2, kT2, vpA, vpB)
            return pair_tiles[pair]

        get_pair(0)
        # keep slices: kt 0..2 upfront (macro-0 masks); the rest are spread
        # one-triple-per-macro inside the loop so the early DMA queues are
        # free for the pair-0 Q/K/V loads
        for kt in range(3):
            nc.gpsimd.dma_start(out=keep_sb[:, kt], in_=keep_ext[:, kt])

        # ---- score batch super-tiles ----
        sc_tiles = {}

        def sc_tile(b):
            if b not in sc_tiles:
                _, ns = batch_slots(b)
                sc_tiles[b] = sc_ps.tile([P, ns, QCH], F32, tag="sc3", name=f"sc_{b}")
            return sc_tiles[b]

        w_tiles = {}

        def issue_qk_slot(s):
            """QK matmul producing head-slot s ([128, 512] scores_T)."""
            i, h01 = divmod(s, 2)
            pr, qc, kt = it_of(i)
            qT2, kT2, _, _ = get_pair(pr)
            b, m = batch_of_slot(s)
            sc = sc_tile(b)
            q0, k0 = qc * QCH, kt * P
            r0 = h01 * DK
            nc.tensor.matmul(
                sc[:, m, :],
                kT2[r0 : r0 + DK, k0 : k0 + P],
                qT2[r0 : r0 + DK, q0 : q0 + QCH],
                start=True,
                stop=True,
                tile_position=(r0, 0),
            )

        def w_tile(mm):
            """per-MACRO w tile [128, 6, 512]: slot s at position s - 6*mm"""
            if mm not in w_tiles:
                ns = min(6, NSLOT - 6 * mm)
                w_tiles[mm] = w_pool.tile(
                    [P, ns, QCH], BF16, tag="w6", name=f"w_{mm}"
                )
            return w_tiles[mm]

        def issue_exp(bb):
            """exp over batch bb's super-tile -> part of the macro w tile."""
            s0, ns = batch_slots(bb)
            if ns <= 0:
                return
            sc = sc_tile(bb)
            w = w_tile(bb // 2)
            a = SPB * (bb % 2)
            nc.scalar.activation(
                w[:, a : a + ns, :], sc, mybir.ActivationFunctionType.Exp, scale=0.125
            )

        def issue_masks(mm, i0, i1):
            """mask multiplies for macro mm's iterations [i0, i1): one DVE
            TT per run of iterations sharing (pr, qc) — kt is consecutive
            within a run, so the keep operand is a clean 4D AP broadcast
            over heads"""
            w = w_tile(mm)
            i1 = min(i1, NITER)
            runs = []
            for i in range(i0, i1):
                pr, qc, kt = it_of(i)
                if runs and runs[-1][0] == (pr, qc):
                    runs[-1][2] += 1
                else:
                    runs.append([(pr, qc), kt, 1, i - 3 * mm])
            for (pr, qc), kt0, nk, ofs in runs:
                q0 = qc * QCH
                wv = w[:, 2 * ofs : 2 * (ofs + nk), :].rearrange(
                    "p (n h) q -> p n h q", h=2
                )
                ks = keep_sb[:, kt0 : kt0 + nk, q0 : q0 + QCH]
                k2 = bass.AP(
                    tensor=ks.tensor,
                    offset=ks.offset,
                    ap=[ks.ap[0], ks.ap[1], [0, 2], ks.ap[2]],
                )
                nc.vector.tensor_mul(wv, wv, k2)

        # ---- acc / epilogue ----
        accs = {}  # block index -> (accA, accB)

        def issue_pv_slot(s):
            i, h01 = divmod(s, 2)
            pr, qc, kt = it_of(i)
            blk = i // NKT
            if kt == 0 and h01 == 0:
                accs[blk] = (
                    acc_ps.tile([DK + 1, QCH], F32, tag="accA", name=f"accA_{blk}"),
                    acc_ps.tile([DK + 1, QCH], F32, tag="accB", name=f"accB_{blk}"),
                )
            acc = accs[blk][h01]
            vp = pair_tiles[pr][2 + h01]
            mm, m = divmod(s, 6)
            nc.tensor.matmul(
                acc,
                vp[:, kt],
                w_tiles[mm][:, m, :],
                start=(kt == 0),
                stop=(kt == NKT - 1),
            )

        # deferred epilogue micro-ops: list of (due_macro, fn)
        pending = []

        def drain_block(blk):
            """Issued INLINE right after the block's last PV and BEFORE the
            next block's kt=0 PV, so the single-buffered acc tiles are never
            reallocated while an un-issued reader is still pending.  Drains
            the whole [65, 512] acc (V rows + sums row) to SBUF f32; the
            sums row is later moved to partition 0 by a tiny SBUF->SBUF DMA
            (engines can't shift partitions; DMA can)."""
            aA, aB = accs[blk]
            dA = ep_pool.tile([DK + 1, QCH], F32, tag="dA", name=f"dA_{blk}")
            dB = ep_pool.tile([DK + 1, QCH], F32, tag="dB", name=f"dB_{blk}")
            nc.vector.tensor_copy(dA, aA)
            nc.vector.tensor_copy(dB, aB)
            return dA, dB

        def schedule_epilogue(blk, mac, st):
            pr, qc = divmod(blk, NQ)
            hA, hB = 2 * pr, 2 * pr + 1
            q0 = qc * QCH

            def p_move(st=st, blk=blk):
                dA, dB = st["d"]
                row = ep_pool.tile([1, 2, QCH], F32, tag="row", name=f"row_{blk}")
                if blk >= NPAIR * NQ - 2:
                    # exps are done by now: the Scalar engine is free and
                    # (unlike the DVE) can shift partitions on a copy
                    nc.scalar.copy(row[:, 0, :], dA[DK : DK + 1, :])
                    nc.scalar.copy(row[:, 1, :], dB[DK : DK + 1, :])
                else:
                    nc.sync.dma_start(out=row[:, 0, :], in_=dA[DK : DK + 1, :])
                    nc.sync.dma_start(out=row[:, 1, :], in_=dB[DK : DK + 1, :])
                st["row"] = row

            def p_bcast(st=st, blk=blk):
                rF = ep_pool.tile([1, 2, QCH], F32, tag="rF", name=f"rF_{blk}")
                nc.vector.reciprocal_approx_fast(rF, st["row"])
                bc = ep_pool.tile([DK, 2, QCH], F32, tag="bc", name=f"bc_{blk}")
                nc.gpsimd.partition_broadcast(bc, rF)
                st["bc"] = bc

            def p_store(st=st, hA=hA, hB=hB, q0=q0, blk=blk):
                dA, dB = st["d"]
                bc = st["bc"]
                oA = ep_pool.tile([DK, QCH], F32, tag="oA", name=f"oA_{blk}")
                oB = ep_pool.tile([DK, QCH], F32, tag="oB", name=f"oB_{blk}")
                nc.vector.tensor_mul(oA, dA[0:DK, :], bc[:, 0, :])
                nc.vector.tensor_mul(oB, dB[0:DK, :], bc[:, 1, :])
                nc.sync.dma_start(out=out_ext[hA, :, q0 : q0 + QCH], in_=oA)
                nc.sync.dma_start(out=out_ext[hB, :, q0 : q0 + QCH], in_=oB)

            pending.append((mac + 1, p_move))
            pending.append((mac + 2, p_bcast))
            pending.append((mac + 3, p_store))

        # ---- macro pipeline ----
        pv = [0]  # PV slot cursor
        # prologue: QK for iterations 0..2 (slots 0..5)
        for s in range(6):
            issue_qk_slot(s)

        for mac in range(NMACRO + 2):
            s_lo = 6 * (mac + 1)
            s_mid = min(s_lo + SPB, NSLOT)
            s_hi = min(s_lo + 6, NSLOT)
            # 1) first exp batch of this macro, then the mask multiply for
            #    the iteration it fully covers.  Each exp MUST be issued
            #    before the QKs that recycle its PSUM buffer (pool bufs=2)
            #    so the WAR dependency is tracked.
            if 2 * mac < NBATCH:
                issue_exp(2 * mac)
            if 3 * mac < NITER:
                issue_masks(mac, 3 * mac, 3 * mac + 1)
            # 2) QK for next macro, first super-tile (recycles batch 2m's
            #    banks; its matmuls wait on exp 2m via the pool WAR dep)
            for s in range(s_lo, s_mid):
                issue_qk_slot(s)
            # 3) second exp batch of this macro, then the macro's mask
            #    multiplies (one big TT per (pr, qc) run)
            if 2 * mac + 1 < NBATCH:
                issue_exp(2 * mac + 1)
            if 3 * mac + 1 < NITER:
                issue_masks(mac, 3 * mac + 1, 3 * mac + 3)
            # spread keep-slice DMAs: triple per macro until loaded
            for kt in range(3 * (mac + 1), min(3 * (mac + 1) + 3, NKT)):
                nc.gpsimd.dma_start(out=keep_sb[:, kt], in_=keep_ext[:, kt])
            # 4) QK for next macro, second super-tile (before the PV block:
            #    the in-order PE queue must not park this refill behind PVs,
            #    or the next macro's second exp starts late)
            for s in range(s_mid, s_hi):
                issue_qk_slot(s)
            # 5) PV stream for iterations up to the previous macro.  When a
            #    block finishes, its (single-buffered) acc banks are drained
            #    inline, and the NEXT block's kt=0 PVs are deferred to the
            #    following macro so the drain is never at the head of the PE
            #    queue when the next macro's QK refills are queued behind it.
            pv_target = min(6 * mac, NSLOT)
            drained_here = False
            while pv[0] < pv_target:
                s = pv[0]
                i, h01 = divmod(s, 2)
                _, _, kt = it_of(i)
                if drained_here and kt == 0 and h01 == 0:
                    break
                issue_pv_slot(s)
                pv[0] += 1
                if h01 == 1 and kt == NKT - 1:
                    blk = i // NKT
                    dA, dB = drain_block(blk)
                    schedule_epilogue(blk, mac, {"d": (dA, dB)})
                    drained_here = True
            # 5) prefetch pair inputs well ahead of their first QK
            pf = 3 * (mac + 5)
            if pf < NITER:
                get_pair(it_of(pf)[0])
            # deferred epilogue ops
            while pending and pending[0][0] <= mac:
                pending.pop(0)[1]()
        while pv[0] < NSLOT:
            s = pv[0]
            i, h01 = divmod(s, 2)
            _, _, kt = it_of(i)
            issue_pv_slot(s)
            pv[0] += 1
            if h01 == 1 and kt == NKT - 1:
                blk = i // NKT
                dA, dB = drain_block(blk)
                schedule_epilogue(blk, NMACRO + 2, {"d": (dA, dB)})
        for _, fn in pending:
            fn()
    nc.finalize()
    return nc


_NC_CACHE = {}


def get_nc():
    if "nc" not in _NC_CACHE:
        _NC_CACHE["nc"] = build_nc()
    return _NC_CACHE["nc"]


def kernel(Q, K, V, mask, _trace=False, _tmpdir=None):
    Q = np.asarray(Q, dtype=np.float32)
    K = np.asarray(K, dtype=np.float32)
    V = np.asarray(V, dtype=np.float32)
    mask = np.asarray(mask)

    in_maps = []
    for c in range(N_CORES):
        b, h0 = c // 2, (c % 2) * HPC
        # [pair, {q,k}, 128, S]: partitions 0:64 = head A dims, 64:128 = head B
        qkt = np.empty((NPAIR, 2, P, S), BF)
        qt = Q[b, h0 : h0 + HPC].transpose(0, 2, 1).reshape(NPAIR, 2 * DK, S)
        kt = K[b, h0 : h0 + HPC].transpose(0, 2, 1).reshape(NPAIR, 2 * DK, S)
        qkt[:, 0] = qt
        qkt[:, 1] = kt
        vp = np.empty((HPC, P, NKT, DK + 1), BF)
        vp[:, :, :, 0:DK] = (
            V[b, h0 : h0 + HPC].reshape(HPC, NKT, P, DK).transpose(0, 2, 1, 3)
        )
        vp[:, :, :, DK] = 1.0
        if c % 2 == 0:
            kp = (~mask[b, 0]).T  # [k, q]
            keep = np.ascontiguousarray(
                kp.reshape(NKT, P, S).transpose(1, 0, 2)
            ).astype(BF)
        in_maps.append({"qkt": qkt, "vp": vp, "keep": keep})

    nc = get_nc()
    res = run_bass_kernel_spmd(
        nc, in_maps, core_ids=list(range(N_CORES)), trace=_trace, tmpdir=_tmpdir
    )
    out = np.empty((B, H, S, DK), np.float32)
    for c in range(N_CORES):
        b, h0 = c // 2, (c % 2) * HPC
        out[b, h0 : h0 + HPC] = np.asarray(res.results[c]["outT"]).transpose(
            0, 2, 1
        )
    if _trace:
        return out, res
    return out


# revision 4
# speedup vs baseline: 1.0137x; 1.0137x over previous
"""Sharded multi-head attention for TRN2 (8 NeuronCores).

Problem: B=4, H=16, S=2048, DK=64 attention with boolean mask [B,1,S,S]
(True entries masked out).  Core c handles batch c//2, heads
(c%2)*8 .. (c%2)*8+8, processed in PAIRS sharing the PE array.

The kernel is organized around keeping the Scalar (ACT) engine — the
bottleneck (exp runs at 1 elem/cycle/lane @ 1.2 GHz, ~262K cycles/core
of pure element work) — saturated with large ACTIVATEs:

  - PSUM layout: score "slots" are per-(iteration, head) [128, 512] f32
    (one bank).  A "batch" super-tile = [128, 3, 512] f32 (3 banks).
    Two batch bufs (6 banks) + single-buffered acc pair (2 banks) = 8.
  - exp runs once per 3-slot batch (FD=1536), halving per-instruction
    overhead vs per-iteration exps.  The 6-bank ring gives each batch a
    full opposite-exp shadow to refill its QK scores: near-zero ACT idle
    in steady state.
  - macro pipeline (macro = 3 iterations = 2 exp batches): per macro the
    PE issues [3 QK slot-matmuls][3 more][6 PV matmuls] so the QK<->PV
    array transitions (full-row LDWEIGHTS can't overlap cross-row-group
    matmuls) amortize, and both QK refills are issued BEFORE the PV
    block — the in-order PE queue must never park a refill behind PVs.
  - QK pairs run concurrently via row tile_position (0,0)/(64,0), each
    head's d=64 contraction in its own row group.
  - mask multiply on DVE (bf16 2x): one TT for the batch-aligned
    iteration right after the first exp, one 2-iteration TT after the
    second (keep slice broadcast over heads via a stride-0 AP dim, and
    over consecutive kt via a 4D AP).
  - PV: acc[v,q] += V'[k,v]^T w per head, V' = [V | ones] so PSUM row 64
    accumulates the softmax denominators.  acc is single-buffered; a
    fast DVE drain (PSUM->SBUF f32), issued inline right after the
    kt=15 PVs, frees the banks before the next block's kt=0 PV needs
    them, keeping the PE-queue bubble at block boundaries small.
  - epilogue off the critical path: the sums row moves to partition 0
    with a tiny SBUF->SBUF DMA (engines can't shift partitions, DMA
    can; the last two blocks use the by-then-idle ACT engine instead),
    reciprocal_approx_fast, gpsimd partition_broadcast, normalize
    multiply, f32 DMA out in [d, q] layout (host un-transposes).
  - a dummy 8-element exp is issued first so the ~2.7us ACT table load
    overlaps the input DMAs; K arrives before Q's later chunks; keep
    mask slices stream in one kt-triple per macro on the gpsimd queue.
"""

import numpy as np
import ml_dtypes
from contextlib import ExitStack

import concourse.bass as bass
import concourse.tile as tile
from concourse import bacc, mybir
from concourse.bass_utils import run_bass_kernel_spmd

B, H, S, DK = 4, 16, 2048, 64
N_CORES = 8
HPC = (B * H) // N_CORES  # heads per core = 8
NPAIR = HPC // 2

P = 128            # k-tile size / partition count
NKT = S // P       # 16 k tiles
QCH = 512          # q chunk per head (slot = [128, 512] f32 = 1 PSUM bank)
NQ = S // QCH      # 4 q chunks

NITER = NPAIR * NQ * NKT          # 256 iterations, flat (pr, qc, kt)
NSLOT = 2 * NITER                 # 512 head-slots
NMACRO = (NITER + 2) // 3         # 86 macros of 3 iterations
SPB = 3                           # slots per exp batch (3 PSUM banks)
NBATCH = 2 * NMACRO


def batch_of_slot(s):
    return divmod(s, SPB)


def batch_slots(bb):
    s0 = SPB * bb
    ns = min(SPB, NSLOT - s0)
    return s0, ns

BF16 = mybir.dt.bfloat16
F32 = mybir.dt.float32
BF = ml_dtypes.bfloat16


def it_of(i):
    pr, r = divmod(i, NQ * NKT)
    qc, kt = divmod(r, NKT)
    return pr, qc, kt


def build_nc():
    nc = bacc.Bacc(None, target_bir_lowering=False)
    # qkt[pair, 0] = [Q_A^T ; Q_B^T] stacked on partitions, [pair, 1] = K
    qkt_ext = nc.declare_dram_parameter("qkt", [NPAIR, 2, P, S], BF16, isOutput=False)
    # vp[h, p, t, :] = [V[h, t*128+p, :], 1.0]
    vp_ext = nc.declare_dram_parameter("vp", [HPC, P, NKT, DK + 1], BF16, isOutput=False)
    # keep[p, t, q] = not mask[q, t*128+p]
    keep_ext = nc.declare_dram_parameter("keep", [P, NKT, S], BF16, isOutput=False)
    # out_T[h, d, q] f32 (host un-transposes)
    out_ext = nc.declare_dram_parameter("outT", [HPC, DK, S], F32, isOutput=True)

    with tile.TileContext(nc) as tc, ExitStack() as ctx:
        singles = ctx.enter_context(tc.tile_pool(name="singles", bufs=1))
        qk_pool = ctx.enter_context(tc.tile_pool(name="qk", bufs=2))
        v_pool = ctx.enter_context(tc.tile_pool(name="vpool", bufs=2))
        w_pool = ctx.enter_context(tc.tile_pool(name="wp", bufs=9))
        ep_pool = ctx.enter_context(tc.tile_pool(name="ep", bufs=3))
        sc_ps = ctx.enter_context(tc.tile_pool(name="scps", bufs=2, space="PSUM"))
        acc_ps = ctx.enter_context(tc.tile_pool(name="accps", bufs=1, space="PSUM"))

        # dummy exp first: forces the ACT table load to overlap input DMAs
        dummy = singles.tile([1, 8], F32)
        nc.vector.memset(dummy, 0.0)
        nc.scalar.activation(dummy, dummy, mybir.ActivationFunctionType.Exp)

        keep_sb = singles.tile([P, NKT, S], BF16)

        pair_tiles = {}

        def get_pair(pair):
            if pair not in pair_tiles:
                hA, hB = 2 * pair, 2 * pair + 1
                qT2 = qk_pool.tile([P, S], BF16, tag="qT2", name=f"qT2_{pair}")
                kT2 = qk_pool.tile([P, S], BF16, tag="kT2", name=f"kT2_{pair}")
                # startup-critical order: K chunk 0 + Q chunk 0 (prologue
                # QKs), V (PVs from macro 1), remaining K (kt>=4), then the
                # remaining Q chunks (qc>=1)
                C4 = S // 4
                nc.sync.dma_start(out=kT2[:, 0:C4], in_=qkt_ext[pair, 1, :, 0:C4])
                nc.sync.dma_start(out=qT2[:, 0:C4], in_=qkt_ext[pair, 0, :, 0:C4])
                vpA = v_pool.tile([P, NKT, DK + 1], BF16, tag="vpA", name=f"vpA_{pair}")
                vpB = v_pool.tile([P, NKT, DK + 1], BF16, tag="vpB", name=f"vpB_{pair}")
                nc.sync.dma_start(out=vpA, in_=vp_ext[hA])
                nc.sync.dma_start(out=vpB, in_=vp_ext[hB])
                for c4 in range(1, 4):
                    cs = slice(c4 * C4, (c4 + 1) * C4)
                    nc.sync.dma_start(out=kT2[:, cs], in_=qkt_ext[pair, 1, :, cs])
                for c4 in range(1, 4):
                    cs = slice(c4 * C4, (c4 + 1) * C4)
                    nc.sync.dma_start(out=qT2[:, cs], in_=qkt_ext[pair, 0, :, cs])
                pair_tiles[pair] = (qT2, kT2, vpA, vpB)
            return pair_tiles[pair]

        get_pair(0)
        # keep slices: kt 0..2 upfront (macro-0 masks); the rest are spread
        # one-triple-per-macro inside the loop so the early DMA queues are
        # free for the pair-0 Q/K/V loads
        for kt in range(3):
            nc.gpsimd.dma_start(out=keep_sb[:, kt], in_=keep_ext[:, kt])

        # ---- score batch super-tiles ----
        sc_tiles = {}

        def sc_tile(b):
            if b not in sc_tiles:
                _, ns = batch_slots(b)
                sc_tiles[b] = sc_ps.tile([P, ns, QCH], F32, tag="sc3", name=f"sc_{b}")
            return sc_tiles[b]

        w_tiles = {}

        def issue_qk_slot(s):
            """QK matmul producing head-slot s ([128, 512] scores_T)."""
            i, h01 = divmod(s, 2)
            pr, qc, kt = it_of(i)
            qT2, kT2, _, _ = get_pair(pr)
            b, m = batch_of_slot(s)
            sc = sc_tile(b)
            q0, k0 = qc * QCH, kt * P
            r0 = h01 * DK
            nc.tensor.matmul(
                sc[:, m, :],
                kT2[r0 : r0 + DK, k0 : k0 + P],
                qT2[r0 : r0 + DK, q0 : q0 + QCH],
                start=True,
                stop=True,
                tile_position=(r0, 0),
            )

        def w_tile(mm):
            """per-MACRO w tile [128, 6, 512]: slot s at position s - 6*mm"""
            if mm not in w_tiles:
                ns = min(6, NSLOT - 6 * mm)
                w_tiles[mm] = w_pool.tile(
                    [P, ns, QCH], BF16, tag="w6", name=f"w_{mm}"
                )
            return w_tiles[mm]

        def issue_exp(bb):
            """exp over batch bb's super-tile -> part of the macro w tile."""
            s0, ns = batch_slots(bb)
            if ns <= 0:
                return
            sc = sc_tile(bb)
            w = w_tile(bb // 2)
            a = SPB * (bb % 2)
            nc.scalar.activation(
                w[:, a : a + ns, :], sc, mybir.ActivationFunctionType.Exp, scale=0.125
            )

        def issue_masks(mm, i0, i1):
            """mask multiplies for macro mm's iterations [i0, i1): one DVE
            TT per run of iterations sharing (pr, qc) — kt is consecutive
            within a run, so the keep operand is a clean 4D AP broadcast
            over heads"""
            w = w_tile(mm)
            i1 = min(i1, NITER)
            runs = []
            for i in range(i0, i1):
                pr, qc, kt = it_of(i)
                if runs and runs[-1][0] == (pr, qc):
                    runs[-1][2] += 1
                else:
                    runs.append([(pr, qc), kt, 1, i - 3 * mm])
            for (pr, qc), kt0, nk, ofs in runs:
                q0 = qc * QCH
                wv = w[:, 2 * ofs : 2 * (ofs + nk), :].rearrange(
                    "p (n h) q -> p n h q", h=2
                )
                ks = keep_sb[:, kt0 : kt0 + nk, q0 : q0 + QCH]
                k2 = bass.AP(
                    tensor=ks.tensor,
                    offset=ks.offset,
                    ap=[ks.ap[0], ks.ap[1], [0, 2], ks.ap[2]],
                )
                nc.vector.tensor_mul(wv, wv, k2)

        # ---- acc / epilogue ----
        accs = {}  # block index -> (accA, accB)

        def issue_pv_slot(s):
            i, h01 = divmod(s, 2)
            pr, qc, kt = it_of(i)
            blk = i // NKT
            if kt == 0 and h01 == 0:
                accs[blk] = (
                    acc_ps.tile([DK + 1, QCH], F32, tag="accA", name=f"accA_{blk}"),
                    acc_ps.tile([DK + 1, QCH], F32, tag="accB", name=f"accB_{blk}"),
                )
            acc = accs[blk][h01]
            vp = pair_tiles[pr][2 + h01]
            mm, m = divmod(s, 6)
            nc.tensor.matmul(
                acc,
                vp[:, kt],
                w_tiles[mm][:, m, :],
                start=(kt == 0),
                stop=(kt == NKT - 1),
            )

        # deferred epilogue micro-ops: list of (due_macro, fn)
        pending = []

        def drain_block(blk):
            """Issued INLINE right after the block's last PV and BEFORE the
            next block's kt=0 PV, so the single-buffered acc tiles are never
            reallocated while an un-issued reader is still pending.  Drains
            the whole [65, 512] acc (V rows + sums row) to SBUF f32; the
            sums row is later moved to partition 0 by a tiny SBUF->SBUF DMA
            (engines can't shift partitions; DMA can)."""
            aA, aB = accs[blk]
            dA = ep_pool.tile([DK + 1, QCH], F32, tag="dA", name=f"dA_{blk}")
            dB = ep_pool.tile([DK + 1, QCH], F32, tag="dB", name=f"dB_{blk}")
            nc.vector.tensor_copy(dA, aA)
            nc.vector.tensor_copy(dB, aB)
            return dA, dB

        def schedule_epilogue(blk, mac, st):
            pr, qc = divmod(blk, NQ)
            hA, hB = 2 * pr, 2 * pr + 1
            q0 = qc * QCH

            def p_move(st=st, blk=blk):
                dA, dB = st["d"]
                row = ep_pool.tile([1, 2, QCH], F32, tag="row", name=f"row_{blk}")
                if blk >= NPAIR * NQ - 1:
                    # exps are done by now: the Scalar engine is free and
                    # (unlike the DVE) can shift partitions on a copy
                    # (last block only: the second-to-last block's epilogue
                    # still overlaps the final exp batches)
                    nc.scalar.copy(row[:, 0, :], dA[DK : DK + 1, :])
                    nc.scalar.copy(row[:, 1, :], dB[DK : DK + 1, :])
                else:
                    nc.sync.dma_start(out=row[:, 0, :], in_=dA[DK : DK + 1, :])
                    nc.sync.dma_start(out=row[:, 1, :], in_=dB[DK : DK + 1, :])
                st["row"] = row

            def p_bcast(st=st, blk=blk):
                rF = ep_pool.tile([1, 2, QCH], F32, tag="rF", name=f"rF_{blk}")
                nc.vector.reciprocal_approx_fast(rF, st["row"])
                bc = ep_pool.tile([DK, 2, QCH], F32, tag="bc", name=f"bc_{blk}")
                nc.gpsimd.partition_broadcast(bc, rF)
                st["bc"] = bc

            def p_store(st=st, hA=hA, hB=hB, q0=q0, blk=blk):
                dA, dB = st["d"]
                bc = st["bc"]
                oA = ep_pool.tile([DK, QCH], F32, tag="oA", name=f"oA_{blk}")
                oB = ep_pool.tile([DK, QCH], F32, tag="oB", name=f"oB_{blk}")
                nc.vector.tensor_mul(oA, dA[0:DK, :], bc[:, 0, :])
                nc.vector.tensor_mul(oB, dB[0:DK, :], bc[:, 1, :])
                nc.sync.dma_start(out=out_ext[hA, :, q0 : q0 + QCH], in_=oA)
                nc.sync.dma_start(out=out_ext[hB, :, q0 : q0 + QCH], in_=oB)

            pending.append((mac + 1, p_move))
            pending.append((mac + 2, p_bcast))
            pending.append((mac + 3, p_store))

        # ---- macro pipeline ----
        pv = [0]  # PV slot cursor
        # prologue: QK for iterations 0..2 (slots 0..5)
        for s in range(6):
            issue_qk_slot(s)

        for mac in range(NMACRO + 2):
            s_lo = 6 * (mac + 1)
            s_mid = min(s_lo + SPB, NSLOT)
            s_hi = min(s_lo + 6, NSLOT)
            # 1) first exp batch of this macro, then the mask multiply for
            #    the iteration it fully covers.  Each exp MUST be issued
            #    before the QKs that recycle its PSUM buffer (pool bufs=2)
            #    so the WAR dependency is tracked.
            if 2 * mac < NBATCH:
                issue_exp(2 * mac)
            if 3 * mac < NITER:
                issue_masks(mac, 3 * mac, 3 * mac + 1)
            # 2) QK for next macro, first super-tile (recycles batch 2m's
            #    banks; its matmuls wait on exp 2m via the pool WAR dep)
            for s in range(s_lo, s_mid):
                issue_qk_slot(s)
            # 3) second exp batch of this macro, then the macro's mask
            #    multiplies (one big TT per (pr, qc) run)
            if 2 * mac + 1 < NBATCH:
                issue_exp(2 * mac + 1)
            if 3 * mac + 1 < NITER:
                issue_masks(mac, 3 * mac + 1, 3 * mac + 3)
            # spread keep-slice DMAs: triple per macro until loaded
            for kt in range(3 * (mac + 1), min(3 * (mac + 1) + 3, NKT)):
                nc.gpsimd.dma_start(out=keep_sb[:, kt], in_=keep_ext[:, kt])
            # 4) QK for next macro, second super-tile (before the PV block:
            #    the in-order PE queue must not park this refill behind PVs,
            #    or the next macro's second exp starts late)
            for s in range(s_mid, s_hi):
                issue_qk_slot(s)
            # 5) PV stream for iterations up to the previous macro.  When a
            #    block finishes, its (single-buffered) acc banks are drained
            #    inline, and the NEXT block's kt=0 PVs are deferred to the
            #    following macro so the drain is never at the head of the PE
            #    queue when the next macro's QK refills are queued behind it.
            pv_target = min(6 * mac, NSLOT)
            drained_here = False
            while pv[0] < pv_target:
                s = pv[0]
                i, h01 = divmod(s, 2)
                _, _, kt = it_of(i)
                if drained_here and kt == 0 and h01 == 0:
                    break
                issue_pv_slot(s)
                pv[0] += 1
                if h01 == 1 and kt == NKT - 1:
                    blk = i // NKT
                    dA, dB = drain_block(blk)
                    schedule_epilogue(blk, mac, {"d": (dA, dB)})
                    drained_here = True
            # 5) prefetch pair inputs well ahead of their first QK
            pf = 3 * (mac + 5)
            if pf < NITER:
                get_pair(it_of(pf)[0])
            # deferred epilogue ops
            while pending and pending[0][0] <= mac:
                pending.pop(0)[1]()
        while pv[0] < NSLOT:
            s = pv[0]
            i, h01 = divmod(s, 2)
            _, _, kt = it_of(i)
            issue_pv_slot(s)
            pv[0] += 1
            if h01 == 1 and kt == NKT - 1:
                blk = i // NKT
                dA, dB = drain_block(blk)
                schedule_epilogue(blk, NMACRO + 2, {"d": (dA, dB)})
        for _, fn in pending:
            fn()
    nc.finalize()
    return nc


_NC_CACHE = {}


def get_nc():
    if "nc" not in _NC_CACHE:
        _NC_CACHE["nc"] = build_nc()
    return _NC_CACHE["nc"]


def kernel(Q, K, V, mask, _trace=False, _tmpdir=None):
    Q = np.asarray(Q, dtype=np.float32)
    K = np.asarray(K, dtype=np.float32)
    V = np.asarray(V, dtype=np.float32)
    mask = np.asarray(mask)

    in_maps = []
    for c in range(N_CORES):
        b, h0 = c // 2, (c % 2) * HPC
        # [pair, {q,k}, 128, S]: partitions 0:64 = head A dims, 64:128 = head B
        qkt = np.empty((NPAIR, 2, P, S), BF)
        qt = Q[b, h0 : h0 + HPC].transpose(0, 2, 1).reshape(NPAIR, 2 * DK, S)
        kt = K[b, h0 : h0 + HPC].transpose(0, 2, 1).reshape(NPAIR, 2 * DK, S)
        qkt[:, 0] = qt
        qkt[:, 1] = kt
        vp = np.empty((HPC, P, NKT, DK + 1), BF)
        vp[:, :, :, 0:DK] = (
            V[b, h0 : h0 + HPC].reshape(HPC, NKT, P, DK).transpose(0, 2, 1, 3)
        )
        vp[:, :, :, DK] = 1.0
        if c % 2 == 0:
            kp = (~mask[b, 0]).T  # [k, q]
            keep = np.ascontiguousarray(
                kp.reshape(NKT, P, S).transpose(1, 0, 2)
            ).astype(BF)
        in_maps.append({"qkt": qkt, "vp": vp, "keep": keep})

    nc = get_nc()
    res = run_bass_kernel_spmd(
        nc, in_maps, core_ids=list(range(N_CORES)), trace=_trace, tmpdir=_tmpdir
    )
    out = np.empty((B, H, S, DK), np.float32)
    for c in range(N_CORES):
        b, h0 = c // 2, (c % 2) * HPC
        out[b, h0 : h0 + HPC] = np.asarray(res.results[c]["outT"]).transpose(
            0, 2, 1
        )
    if _trace:
        return out, res
    return out
